# revision 4
# baseline (speedup 1.0000x reference)
"""Multi-head dilated sliding-window attention (window=129, dil=1) on 8 TRN2 cores.

Sharding: sequence-parallel. Each core computes 256 query rows (N=2048 / 8),
with a 64-row K/V halo on each side (zero-padded at the sequence edges).
Weights are replicated (streamed to SBUF, bf16).

Band-softmax identity (reference softmaxes the FULL row with zeros outside
the band):
    out_i = (sum_band (e^{s_ij} - 1) V_j + sum_all V_j) / (sum_band (e^{s_ij} - 1) + N)
with V_raw = x@Wv (bv folded into bo2 = bv@Wo + bo on host), bk added as a
per-partition scalar on the K^T copy (padding keys are excluded by per-core
edge-masked band masks), and sum_all V_j = (sum_n x_n)@Wv computed on host.

Host-side layout prep: x arrives pre-transposed (xT[e,seq]), Wq/Wk arrive
column-block-major so round db's weight slice is one contiguous DMA.

Schedule: one DMA issue stream on the Sync engine in consumption order
(issue cost ~650ns each serializes on Sync; transfers fan out over all 16
DMA engines at ~344 GB/s aggregate). V = x@Wv streams et-major right behind
the DMA. Then 8 rounds, one head-pair each: Q^T/K^T projections, previous
round's PV + epilogue, scores + exp chain (exp on ACT, -1 on GpSimd, mask
mul on DVE). Output projection runs as one dense block at the end from the
accumulated A^T tiles.
"""

import numpy as np
import ml_dtypes
from contextlib import ExitStack

import concourse.bass as bass
import concourse.tile as tile
from concourse import bacc, mybir
from concourse.bass_utils import run_bass_kernel_spmd

F32 = mybir.dt.float32
BF16 = mybir.dt.bfloat16
NPBF16 = ml_dtypes.bfloat16
N, E, H, D = 2048, 1024, 16, 64
R = N // 8          # 256 query rows per core
HALO = R + 128      # 384 K/V rows per core
NQB = R // 128      # query blocks per core


def build_graph():
    nc = bacc.Bacc("TRN2", target_bir_lowering=False, debug=False, num_devices=8)

    xT_d = nc.declare_dram_parameter("xT", [128, 8 * HALO], BF16, isOutput=False)
    wqk_d = nc.declare_dram_parameter("wqk", [E, 2048], BF16, isOutput=False)
    wv_d = nc.declare_dram_parameter("Wv", [E, H * D], BF16, isOutput=False)
    wo_d = nc.declare_dram_parameter("Wo", [H * D, E], BF16, isOutput=False)
    sm_d = nc.declare_dram_parameter("smalls", [128, 528], F32, isOutput=False)
    id_d = nc.declare_dram_parameter("ident", [128, 128], BF16, isOutput=False)
    bc_d = nc.declare_dram_parameter("biascat", [1, H * (D + 1)], BF16,
                                     isOutput=False)
    bo2_d = nc.declare_dram_parameter("bo2t", [128, E], F32, isOutput=False)
    out_d = nc.declare_dram_parameter("out", [R, E], F32, isOutput=True)

    with tile.TileContext(nc) as tc, ExitStack() as ctx:
        const = ctx.enter_context(tc.tile_pool(name="const", bufs=1))
        pers = ctx.enter_context(tc.tile_pool(name="pers", bufs=1))
        qtp = ctx.enter_context(tc.tile_pool(name="qtp", bufs=2))
        ktp = ctx.enter_context(tc.tile_pool(name="ktp", bufs=2))
        epool = ctx.enter_context(tc.tile_pool(name="epool", bufs=3))
        ppool = ctx.enter_context(tc.tile_pool(name="ppool", bufs=4))
        apool = ctx.enter_context(tc.tile_pool(name="apool", bufs=2))
        zpool = ctx.enter_context(tc.tile_pool(name="zpool", bufs=8))
        obpool = ctx.enter_context(tc.tile_pool(name="obpool", bufs=2))
        psum = ctx.enter_context(tc.tile_pool(name="psum", bufs=8, space="PSUM"))

        def ps(shape, dt=F32):
            return psum.tile(shape, dt, tag="ps", name="pst")

        # ---- SBUF tiles -------------------------------------------------
        xT = const.tile([128, 8, HALO], BF16, tag="xT")
        wqk_t = [const.tile([128, 2, 8, 128], BF16, tag=f"wqk{db}", name="wqkt")
                 for db in range(8)]
        wv_t = [const.tile([128, E], BF16, tag=f"wv{et}", name="wvt") for et in range(8)]
        wo_t = [const.tile([128, E], BF16, tag=f"wo{db}", name="wot") for db in range(8)]
        smalls = const.tile([128, 528], F32, tag="smalls")
        bq_sb = smalls[:, 0:8]
        bk_sb = smalls[:, 8:16]
        m4 = smalls[:, 16:528]
        ident = const.tile([128, 128], BF16, tag="ident")
        biascat = const.tile([1, H, D + 1], BF16, tag="biascat")
        bo2 = const.tile([128, E], F32, tag="bo2")
        ones_sb = const.tile([1, 128], BF16, tag="ones")
        nc.vector.memset(ones_sb[:], 1.0)

        Vaug = pers.tile([128, 3, H, D + 1], BF16, tag="Vaug")
        AT = pers.tile([128, 8, R], BF16, tag="AT")  # [d_p, db, q]

        # ---- single DMA issue stream on Sync, consumption order ---------
        for c in range(4):  # xT in 4 et-pair chunks
            nc.sync.dma_start(xT[:, 2 * c:2 * c + 2, :],
                              xT_d[:, 2 * c * HALO:(2 * c + 2) * HALO])
        for et in range(5):
            nc.sync.dma_start(wv_t[et][:], wv_d[et * 128:(et + 1) * 128, :])
        nc.sync.dma_start(wqk_t[0][:], wqk_d[0:128, :])
        for et in range(5, 8):
            nc.sync.dma_start(wv_t[et][:], wv_d[et * 128:(et + 1) * 128, :])
        nc.sync.dma_start(smalls[:], sm_d[:, :])
        nc.sync.dma_start(ident[:], id_d[:, :])
        nc.sync.dma_start(biascat[:], bc_d[:, :])
        for db in range(1, 8):
            nc.sync.dma_start(wqk_t[db][:], wqk_d[db * 128:(db + 1) * 128, :])
        for db in range(8):
            nc.sync.dma_start(wo_t[db][:], wo_d[db * 128:(db + 1) * 128, :])
        nc.sync.dma_start(bo2[:], bo2_d[:, :])

        # ---- PE clock warm-up during the x/wv DMA wait ------------------
        wu = const.tile([128, 128], BF16, tag="wu")
        nc.vector.memset(wu[:], 0.0)
        wups = psum.tile([128, 128], F32, tag="ps", name="wups")
        for _ in range(24):
            nc.tensor.matmul(wups[:], wu[:], wu[:], start=True, stop=True)

        # ---- V projection, et-major (streams behind the DMA) ------------
        vps = [psum.tile([128, 512], F32, tag="ps", name="vps") for _ in range(6)]  # st*2 + hf
        for et in range(8):
            for st in range(3):
                for hf in range(2):
                    nc.tensor.matmul(vps[st * 2 + hf][:],
                                     xT[:, et, st * 128:(st + 1) * 128],
                                     wv_t[et][:, hf * 512:(hf + 1) * 512],
                                     start=(et == 0), stop=(et == 7))
        for st in range(3):
            for hf in range(2):
                src = vps[st * 2 + hf][:].rearrange("p (h d) -> p h d", d=D)
                dst = Vaug[:, st, hf * 8:(hf + 1) * 8, 0:D]
                if hf == 0:
                    nc.scalar.copy(dst, src)
                else:
                    nc.vector.tensor_copy(dst, src)
        nc.vector.memset(Vaug[:, :, :, D:D + 1], 1.0)

        # ---- fused rounds: one head-pair db per round --------------------
        def pv_mms(pr):
            db, ptl = pr
            pv = ps([128, 2, 2, 65])  # [qblk, i, D+1]
            first = True
            for qblk in range(NQB):
                for i in range(2):
                    for cblk in range(2):
                        quad = qblk * 2 + cblk
                        nc.tensor.matmul(pv[:, qblk, i, :],
                                         ptl[i][:, quad * 128:(quad + 1) * 128],
                                         Vaug[:, qblk + cblk, 2 * db + i, :],
                                         start=first, stop=False)
                        first = False
            for qblk in range(NQB):
                for i in range(2):
                    nc.tensor.matmul(pv[:, qblk, i, :], ones_sb[0:1, :],
                                     biascat[0:1, 2 * db + i, :], start=False,
                                     stop=(qblk == 1 and i == 1))
            return pv

        def pv_epilogue(db, pv):
            asc = apool.tile([128, 2, 128], BF16, tag="asc", name="asc")
            for qblk in range(NQB):
                for i in range(2):
                    zin = zpool.tile([128, 1], F32, tag="z", name="zin")
                    nc.vector.reciprocal(zin[:], pv[:, qblk, i, 64:65])
                    nc.scalar.activation(asc[:, qblk, i * 64:(i + 1) * 64],
                                         pv[:, qblk, i, 0:64],
                                         mybir.ActivationFunctionType.Copy,
                                         scale=zin[:])
            return asc

        def at_transposes(db, asc):
            for qblk in range(NQB):
                tp = ps([128, 128], BF16)
                nc.tensor.transpose(tp[:], asc[:, qblk, :], ident[:])
                nc.vector.tensor_copy(AT[:, db, qblk * 128:(qblk + 1) * 128],
                                      tp[:])

        prev = None  # (db, ptiles)
        for r in range(9):
            if r < 8:
                db = r
                qp = ps([128, R])
                for et in range(8):
                    nc.tensor.matmul(qp[:], wqk_t[db][:, 0, et, :],
                                     xT[:, et, 64:64 + R],
                                     start=(et == 0), stop=(et == 7))
                qt = qtp.tile([128, R], BF16, tag="qt", name="qt")
                nc.scalar.add(qt[:], qp[:], bq_sb[:, db:db + 1])
                kp = ps([128, HALO])
                for et in range(8):
                    nc.tensor.matmul(kp[:], wqk_t[db][:, 1, et, :],
                                     xT[:, et, :],
                                     start=(et == 0), stop=(et == 7))
                kt = ktp.tile([128, HALO], BF16, tag="kt", name="kt")
                nc.scalar.add(kt[:], kp[:], bk_sb[:, db:db + 1])
                if prev is not None:
                    pvp = pv_mms(prev)
                ptl = {}
                for i in range(2):
                    rr = i * 64
                    sp = ps([128, 512])
                    for quad in range(4):
                        qblk, cblk = quad // 2, quad % 2
                        nc.tensor.matmul(
                            sp[:, quad * 128:(quad + 1) * 128],
                            kt[rr:rr + 64,
                               (qblk + cblk) * 128:(qblk + cblk + 1) * 128],
                            qt[rr:rr + 64, qblk * 128:(qblk + 1) * 128],
                            start=(quad == 0), stop=(quad == 3))
                    et_ = epool.tile([128, 512], F32, tag="e", name="et_")
                    nc.scalar.activation(et_[:], sp[:],
                                         mybir.ActivationFunctionType.Exp)
                    nc.gpsimd.tensor_scalar_add(et_[:], et_[:], -1.0)
                    pt = ppool.tile([128, 512], BF16, tag="p", name="pt")
                    nc.vector.tensor_mul(pt[:], et_[:], m4)
                    ptl[i] = pt
                if prev is not None:
                    pasc = pv_epilogue(prev[0], pvp)
                    at_transposes(prev[0], pasc)
                prev = (db, ptl)
            else:
                pvp = pv_mms(prev)
                pasc = pv_epilogue(prev[0], pvp)
                at_transposes(prev[0], pasc)

        # ---- output projection: dense block at the end -------------------
        # O[q, :] = sum_db AT[:, db, q]^T @ Wo[db*128:(db+1)*128, :] + bo2
        for qblk in range(NQB):
            ops = [psum.tile([128, 512], F32, tag="ps", name="ops") for _ in range(2)]
            for db in range(8):
                for hf in range(2):
                    nc.tensor.matmul(ops[hf][:],
                                     AT[:, db, qblk * 128:(qblk + 1) * 128],
                                     wo_t[db][:, hf * 512:(hf + 1) * 512],
                                     start=(db == 0), stop=(db == 7))
            ob = obpool.tile([128, E], F32, tag="ob", name="ob")
            for hf in range(2):
                nc.vector.tensor_tensor(ob[:, hf * 512:(hf + 1) * 512],
                                        ops[hf][:], bo2[:, hf * 512:(hf + 1) * 512],
                                        mybir.AluOpType.add)
            nc.sync.dma_start(out_d[qblk * 128:(qblk + 1) * 128, :], ob[:])

    nc.compile()
    return nc


_NC = None


def get_nc():
    global _NC
    if _NC is None:
        _NC = build_graph()
    return _NC


def make_in_maps(x, Wq, bq, Wk, bk, Wv, bv, Wo, bo):
    f = lambda a: np.ascontiguousarray(np.asarray(a, dtype=np.float32))
    bf = lambda a: np.ascontiguousarray(
        np.asarray(a, dtype=np.float32).astype(NPBF16))
    x2 = f(x).reshape(N, E)
    Wqf, Wkf, Wvf, Wof = f(Wq), f(Wk), f(Wv), f(Wo)
    # Wq/Wk column-block-major: wqk[db, p, qk, et, j] = W[et*128+p, db*128+j]
    wqT = Wqf.reshape(8, 128, 8, 128).transpose(2, 1, 0, 3)
    wkT = Wkf.reshape(8, 128, 8, 128).transpose(2, 1, 0, 3)
    wqk = np.stack([wqT, wkT], axis=2).reshape(E, 2048)
    # band masks per quadrant [m0 | m1 | m0 | m1]
    ci = np.arange(128, dtype=np.float32)[:, None]  # key index c (partitions)
    qi = np.arange(128, dtype=np.float32)[None, :]  # query index q (free)
    m0 = (ci >= qi).astype(np.float32)
    m1 = (ci <= qi).astype(np.float32)
    mask4 = np.concatenate([m0, m1, m0, m1], axis=1)
    # biascat rows: [SV_h (64) | N] per head, SV = (sum_n x_n) @ Wv
    SV = x2.sum(0, dtype=np.float32) @ Wvf
    bcat = np.zeros((H, D + 1), np.float32)
    bcat[:, 0:D] = SV.reshape(H, D)
    bcat[:, D] = float(N)
    bo2 = f(bv) @ Wof + f(bo)
    common = {
        "wqk": bf(wqk),
        "Wv": bf(Wvf), "Wo": bf(Wof),
        "ident": np.eye(128, dtype=np.float32).astype(NPBF16),
        "biascat": bcat.reshape(1, H * (D + 1)).astype(NPBF16),
        "bo2t": np.ascontiguousarray(np.tile(bo2[None, :], (128, 1))),
    }
    in_maps = []
    for c in range(8):
        r0 = c * R
        xh = np.zeros((HALO, E), np.float32)
        lo, hi = r0 - 64, r0 + R + 64
        slo, shi = max(lo, 0), min(hi, N)
        xh[slo - lo: shi - lo] = x2[slo:shi]
        xTh = xh.T.reshape(8, 128, HALO).transpose(1, 0, 2).reshape(128, 8 * HALO)
        m4c = mask4.copy()
        if c == 0:    # halo rows 0:64 are padding, used only by quad 0
            m4c[0:64, 0:128] = 0.0
        if c == 7:    # halo rows 320:384 are padding, used only by quad 3
            m4c[64:128, 384:512] = 0.0
        sm = np.zeros((128, 528), np.float32)
        sm[:, 0:8] = f(bq).reshape(8, 128).T
        sm[:, 8:16] = f(bk).reshape(8, 128).T
        sm[:, 16:528] = m4c
        in_maps.append({**common,
                        "xT": np.ascontiguousarray(xTh.astype(NPBF16)),
                        "smalls": np.ascontiguousarray(sm)})
    return in_maps


def kernel(x, Wq, bq, Wk, bk, Wv, bv, Wo, bo, _trace=False, _trace_kwargs=None):
    nc = get_nc()
    in_maps = make_in_maps(x, Wq, bq, Wk, bk, Wv, bv, Wo, bo)
    res = run_bass_kernel_spmd(nc, in_maps, list(range(8)), trace=_trace,
                               **(_trace_kwargs or {}))
    out = np.concatenate([res.results[c]["out"] for c in range(8)], axis=0)
    kernel.last_result = res
    return out[None].astype(np.float32)


# revision 5
# speedup vs baseline: 2.0239x; 2.0239x over previous
"""Multi-head dilated sliding-window attention (window=129, dil=1) on 8 TRN2 cores.

Sharding: sequence-parallel. Each core computes 256 query rows (N=2048 / 8),
with a 64-row K/V halo on each side (zero-padded at the sequence edges).
Weights are replicated (streamed to SBUF, bf16).

Band-softmax identity (reference softmaxes the FULL row with zeros outside
the band):
    out_i = (sum_band (e^{s_ij} - 1) V_j + sum_all V_j) / (sum_band (e^{s_ij} - 1) + N)
with V_raw = x@Wv (bv folded into bo2 = bv@Wo + bo on host), bk added as a
per-partition scalar on the K^T copy (padding keys are excluded by per-core
edge-masked band masks), and sum_all V_j = (sum_n x_n)@Wv computed on host.

Host-side layout prep: x arrives pre-transposed (xT[e,seq]), Wq/Wk arrive
column-block-major so round db's weight slice is one contiguous DMA.

Schedule: one DMA issue stream on the Sync engine in consumption order
(issue cost ~650ns each serializes on Sync; transfers fan out over all 16
DMA engines at ~344 GB/s aggregate). V = x@Wv streams et-major right behind
the DMA. Then 8 rounds, one head-pair each: Q^T/K^T projections, previous
round's PV + epilogue, scores + exp chain (exp on ACT, -1 on GpSimd, mask
mul on DVE). Output projection runs as one dense block at the end from the
accumulated A^T tiles.
"""

import numpy as np
import ml_dtypes
from contextlib import ExitStack

import concourse.bass as bass
import concourse.tile as tile
from concourse import bacc, mybir
from concourse.bass_utils import run_bass_kernel_spmd

F32 = mybir.dt.float32
BF16 = mybir.dt.bfloat16
NPBF16 = ml_dtypes.bfloat16
N, E, H, D = 2048, 1024, 16, 64
R = N // 8          # 256 query rows per core
HALO = R + 128      # 384 K/V rows per core
NQB = R // 128      # query blocks per core


def build_graph():
    nc = bacc.Bacc("TRN2", target_bir_lowering=False, debug=False, num_devices=8)

    xT_d = nc.declare_dram_parameter("xT", [128, 8 * HALO], BF16, isOutput=False)
    wqk_d = nc.declare_dram_parameter("wqk", [E, 2048], BF16, isOutput=False)
    wv_d = nc.declare_dram_parameter("Wv", [E, H * D], BF16, isOutput=False)
    wo_d = nc.declare_dram_parameter("Wo", [H * D, E], BF16, isOutput=False)
    sm_d = nc.declare_dram_parameter("smalls", [128, 16], F32, isOutput=False)
    mb_d = nc.declare_dram_parameter("maskb", [128, 640], BF16, isOutput=False)
    bc_d = nc.declare_dram_parameter("biascat", [1, H * (D + 1)], BF16,
                                     isOutput=False)
    bo2_d = nc.declare_dram_parameter("bo2t", [128, E], F32, isOutput=False)
    out_d = nc.declare_dram_parameter("out", [R, E], F32, isOutput=True)

    with tile.TileContext(nc) as tc, ExitStack() as ctx:
        const = ctx.enter_context(tc.tile_pool(name="const", bufs=1))
        pers = ctx.enter_context(tc.tile_pool(name="pers", bufs=1))
        qtp = ctx.enter_context(tc.tile_pool(name="qtp", bufs=2))
        ktp = ctx.enter_context(tc.tile_pool(name="ktp", bufs=2))
        epool = ctx.enter_context(tc.tile_pool(name="epool", bufs=3))
        ppool = ctx.enter_context(tc.tile_pool(name="ppool", bufs=4))
        apool = ctx.enter_context(tc.tile_pool(name="apool", bufs=2))
        zpool = ctx.enter_context(tc.tile_pool(name="zpool", bufs=8))
        obpool = ctx.enter_context(tc.tile_pool(name="obpool", bufs=2))
        psum = ctx.enter_context(tc.tile_pool(name="psum", bufs=8, space="PSUM"))

        def ps(shape, dt=F32):
            return psum.tile(shape, dt, tag="ps", name="pst")

        # ---- SBUF tiles -------------------------------------------------
        xT = const.tile([128, 8, HALO], BF16, tag="xT")
        wqk_t = [const.tile([128, 2, 8, 128], BF16, tag=f"wqk{db}", name="wqkt")
                 for db in range(8)]
        wv_t = [const.tile([128, E], BF16, tag=f"wv{et}", name="wvt") for et in range(8)]
        wo_t = [const.tile([128, E], BF16, tag=f"wo{db}", name="wot") for db in range(8)]
        smalls = const.tile([128, 16], F32, tag="smalls")
        bq_sb = smalls[:, 0:8]
        bk_sb = smalls[:, 8:16]
        maskb = const.tile([128, 640], BF16, tag="maskb")
        m4 = maskb[:, 0:512]
        ident = maskb[:, 512:640]
        biascat = const.tile([1, H, D + 1], BF16, tag="biascat")
        bo2 = const.tile([128, E], F32, tag="bo2")
        ones_sb = const.tile([1, 128], BF16, tag="ones")
        nc.vector.memset(ones_sb[:], 1.0)

        Vaug = pers.tile([128, 3, H, D + 1], BF16, tag="Vaug")
        AT = pers.tile([128, 8, R], BF16, tag="AT")  # [d_p, db, q]

        # ---- single DMA issue stream on Sync, consumption order ---------
        for c in range(4):  # xT in 4 et-pair chunks
            nc.sync.dma_start(xT[:, 2 * c:2 * c + 2, :],
                              xT_d[:, 2 * c * HALO:(2 * c + 2) * HALO])
        for et in range(5):
            nc.sync.dma_start(wv_t[et][:], wv_d[et * 128:(et + 1) * 128, :])
        nc.sync.dma_start(wqk_t[0][:], wqk_d[0:128, :])
        for et in range(5, 8):
            nc.sync.dma_start(wv_t[et][:], wv_d[et * 128:(et + 1) * 128, :])
        nc.sync.dma_start(smalls[:], sm_d[:, :])
        nc.sync.dma_start(maskb[:], mb_d[:, :])
        nc.sync.dma_start(biascat[:], bc_d[:, :])
        for db in range(1, 8):
            nc.sync.dma_start(wqk_t[db][:], wqk_d[db * 128:(db + 1) * 128, :])
        for db in range(8):
            nc.sync.dma_start(wo_t[db][:], wo_d[db * 128:(db + 1) * 128, :])
        nc.sync.dma_start(bo2[:], bo2_d[:, :])

        # ---- PE clock warm-up during the x/wv DMA wait ------------------
        wu = const.tile([128, 128], BF16, tag="wu")
        nc.vector.memset(wu[:], 0.0)
        wups = psum.tile([128, 128], F32, tag="ps", name="wups")
        for _ in range(24):
            nc.tensor.matmul(wups[:], wu[:], wu[:], start=True, stop=True)

        # ---- V projection, et-major (streams behind the DMA) ------------
        vps = [psum.tile([128, 512], F32, tag="ps", name="vps") for _ in range(6)]  # st*2 + hf
        for et in range(8):
            for st in range(3):
                for hf in range(2):
                    nc.tensor.matmul(vps[st * 2 + hf][:],
                                     xT[:, et, st * 128:(st + 1) * 128],
                                     wv_t[et][:, hf * 512:(hf + 1) * 512],
                                     start=(et == 0), stop=(et == 7))
        for st in range(3):
            for hf in range(2):
                src = vps[st * 2 + hf][:].rearrange("p (h d) -> p h d", d=D)
                dst = Vaug[:, st, hf * 8:(hf + 1) * 8, 0:D]
                if hf == 0:
                    nc.scalar.copy(dst, src)
                else:
                    nc.vector.tensor_copy(dst, src)
        nc.vector.memset(Vaug[:, :, :, D:D + 1], 1.0)

        # ---- fused rounds: one head-pair db per round --------------------
        def pv_mms(pr):
            db, ptl = pr
            pv = ps([128, 2, 2, 65])  # [qblk, i, D+1]
            first = True
            for qblk in range(NQB):
                for i in range(2):
                    for cblk in range(2):
                        quad = qblk * 2 + cblk
                        nc.tensor.matmul(pv[:, qblk, i, :],
                                         ptl[i][:, quad * 128:(quad + 1) * 128],
                                         Vaug[:, qblk + cblk, 2 * db + i, :],
                                         start=first, stop=False)
                        first = False
            for qblk in range(NQB):
                for i in range(2):
                    nc.tensor.matmul(pv[:, qblk, i, :], ones_sb[0:1, :],
                                     biascat[0:1, 2 * db + i, :], start=False,
                                     stop=(qblk == 1 and i == 1))
            return pv

        def pv_epilogue(db, pv):
            asc = apool.tile([128, 2, 128], BF16, tag="asc", name="asc")
            for qblk in range(NQB):
                for i in range(2):
                    zin = zpool.tile([128, 1], F32, tag="z", name="zin")
                    nc.vector.reciprocal(zin[:], pv[:, qblk, i, 64:65])
                    nc.scalar.activation(asc[:, qblk, i * 64:(i + 1) * 64],
                                         pv[:, qblk, i, 0:64],
                                         mybir.ActivationFunctionType.Copy,
                                         scale=zin[:])
            return asc

        def at_transposes(db, asc):
            for qblk in range(NQB):
                tp = ps([128, 128], BF16)
                nc.tensor.transpose(tp[:], asc[:, qblk, :], ident[:])
                nc.vector.tensor_copy(AT[:, db, qblk * 128:(qblk + 1) * 128],
                                      tp[:])

        prev = None  # (db, ptiles)
        for r in range(9):
            if r < 8:
                db = r
                qp = ps([128, R])
                for et in range(8):
                    nc.tensor.matmul(qp[:], wqk_t[db][:, 0, et, :],
                                     xT[:, et, 64:64 + R],
                                     start=(et == 0), stop=(et == 7))
                qt = qtp.tile([128, R], BF16, tag="qt", name="qt")
                nc.scalar.add(qt[:], qp[:], bq_sb[:, db:db + 1])
                kp = ps([128, HALO])
                for et in range(8):
                    nc.tensor.matmul(kp[:], wqk_t[db][:, 1, et, :],
                                     xT[:, et, :],
                                     start=(et == 0), stop=(et == 7))
                kt = ktp.tile([128, HALO], BF16, tag="kt", name="kt")
                nc.scalar.add(kt[:], kp[:], bk_sb[:, db:db + 1])
                if prev is not None:
                    pvp = pv_mms(prev)
                ptl = {}
                for i in range(2):
                    rr = i * 64
                    sp = ps([128, 512])
                    for quad in range(4):
                        qblk, cblk = quad // 2, quad % 2
                        nc.tensor.matmul(
                            sp[:, quad * 128:(quad + 1) * 128],
                            kt[rr:rr + 64,
                               (qblk + cblk) * 128:(qblk + cblk + 1) * 128],
                            qt[rr:rr + 64, qblk * 128:(qblk + 1) * 128],
                            start=(quad == 0), stop=(quad == 3))
                    et_ = epool.tile([128, 512], BF16, tag="e", name="et_")
                    nc.scalar.activation(et_[:], sp[:],
                                         mybir.ActivationFunctionType.Exp)
                    nc.vector.tensor_scalar_add(et_[:], et_[:], -1.0)
                    pt = ppool.tile([128, 512], BF16, tag="p", name="pt")
                    nc.vector.tensor_mul(pt[:], et_[:], m4)
                    ptl[i] = pt
                if prev is not None:
                    pasc = pv_epilogue(prev[0], pvp)
                    at_transposes(prev[0], pasc)
                prev = (db, ptl)
            else:
                pvp = pv_mms(prev)
                pasc = pv_epilogue(prev[0], pvp)
                at_transposes(prev[0], pasc)

        # ---- output projection: dense block at the end -------------------
        # O[q, :] = sum_db AT[:, db, q]^T @ Wo[db*128:(db+1)*128, :] + bo2
        for qblk in range(NQB):
            ops = [psum.tile([128, 512], F32, tag="ps", name="ops") for _ in range(2)]
            for db in range(8):
                for hf in range(2):
                    nc.tensor.matmul(ops[hf][:],
                                     AT[:, db, qblk * 128:(qblk + 1) * 128],
                                     wo_t[db][:, hf * 512:(hf + 1) * 512],
                                     start=(db == 0), stop=(db == 7))
            ob = obpool.tile([128, E], F32, tag="ob", name="ob")
            for hf in range(2):
                nc.vector.tensor_tensor(ob[:, hf * 512:(hf + 1) * 512],
                                        ops[hf][:], bo2[:, hf * 512:(hf + 1) * 512],
                                        mybir.AluOpType.add)
            nc.sync.dma_start(out_d[qblk * 128:(qblk + 1) * 128, :], ob[:])

    nc.compile()
    return nc


_NC = None


def get_nc():
    global _NC
    if _NC is None:
        _NC = build_graph()
    return _NC


def make_in_maps(x, Wq, bq, Wk, bk, Wv, bv, Wo, bo):
    f = lambda a: np.ascontiguousarray(np.asarray(a, dtype=np.float32))
    bf = lambda a: np.ascontiguousarray(
        np.asarray(a, dtype=np.float32).astype(NPBF16))
    x2 = f(x).reshape(N, E)
    Wqf, Wkf, Wvf, Wof = f(Wq), f(Wk), f(Wv), f(Wo)
    # Wq/Wk column-block-major: wqk[db, p, qk, et, j] = W[et*128+p, db*128+j]
    wqT = Wqf.reshape(8, 128, 8, 128).transpose(2, 1, 0, 3)
    wkT = Wkf.reshape(8, 128, 8, 128).transpose(2, 1, 0, 3)
    wqk = np.stack([wqT, wkT], axis=2).reshape(E, 2048)
    # band masks per quadrant [m0 | m1 | m0 | m1]
    ci = np.arange(128, dtype=np.float32)[:, None]  # key index c (partitions)
    qi = np.arange(128, dtype=np.float32)[None, :]  # query index q (free)
    m0 = (ci >= qi).astype(np.float32)
    m1 = (ci <= qi).astype(np.float32)
    mask4 = np.concatenate([m0, m1, m0, m1], axis=1)
    # biascat rows: [SV_h (64) | N] per head, SV = (sum_n x_n) @ Wv
    SV = x2.sum(0, dtype=np.float32) @ Wvf
    bcat = np.zeros((H, D + 1), np.float32)
    bcat[:, 0:D] = SV.reshape(H, D)
    bcat[:, D] = float(N)
    bo2 = f(bv) @ Wof + f(bo)
    common = {
        "wqk": bf(wqk),
        "Wv": bf(Wvf), "Wo": bf(Wof),
        "biascat": bcat.reshape(1, H * (D + 1)).astype(NPBF16),
        "bo2t": np.ascontiguousarray(np.tile(bo2[None, :], (128, 1))),
    }
    in_maps = []
    for c in range(8):
        r0 = c * R
        xh = np.zeros((HALO, E), np.float32)
        lo, hi = r0 - 64, r0 + R + 64
        slo, shi = max(lo, 0), min(hi, N)
        xh[slo - lo: shi - lo] = x2[slo:shi]
        xTh = xh.T.reshape(8, 128, HALO).transpose(1, 0, 2).reshape(128, 8 * HALO)
        m4c = mask4.copy()
        if c == 0:    # halo rows 0:64 are padding, used only by quad 0
            m4c[0:64, 0:128] = 0.0
        if c == 7:    # halo rows 320:384 are padding, used only by quad 3
            m4c[64:128, 384:512] = 0.0
        sm = np.zeros((128, 16), np.float32)
        sm[:, 0:8] = f(bq).reshape(8, 128).T
        sm[:, 8:16] = f(bk).reshape(8, 128).T
        mb = np.zeros((128, 640), np.float32)
        mb[:, 0:512] = m4c
        mb[:, 512:640] = np.eye(128, dtype=np.float32)
        in_maps.append({**common,
                        "xT": np.ascontiguousarray(xTh.astype(NPBF16)),
                        "smalls": np.ascontiguousarray(sm),
                        "maskb": np.ascontiguousarray(mb.astype(NPBF16))})
    return in_maps


def kernel(x, Wq, bq, Wk, bk, Wv, bv, Wo, bo, _trace=False, _trace_kwargs=None):
    nc = get_nc()
    in_maps = make_in_maps(x, Wq, bq, Wk, bk, Wv, bv, Wo, bo)
    res = run_bass_kernel_spmd(nc, in_maps, list(range(8)), trace=_trace,
                               **(_trace_kwargs or {}))
    out = np.concatenate([res.results[c]["out"] for c in range(8)], axis=0)
    kernel.last_result = res
    return out[None].astype(np.float32)


# revision 6
# speedup vs baseline: 2.2039x; 1.0889x over previous
"""Multi-head dilated sliding-window attention (window=129, dil=1) on 8 TRN2 cores.

Sharding: sequence-parallel. Each core computes 256 query rows (N=2048 / 8),
with a 64-row K/V halo on each side (zero-padded at the sequence edges).
Weights are replicated (streamed to SBUF, bf16).

Band-softmax identity (reference softmaxes the FULL row with zeros outside
the band):
    out_i = (sum_band (e^{s_ij} - 1) V_j + sum_all V_j) / (sum_band (e^{s_ij} - 1) + N)
with V_raw = x@Wv (bv folded into bo2 = bv@Wo + bo on host), bk added as a
per-partition scalar on the K^T copy (padding keys are excluded by per-core
edge-masked band masks), and sum_all V_j = (sum_n x_n)@Wv computed on host.

Host-side layout prep: x arrives pre-transposed (xT[e,seq]), Wq/Wk arrive
column-block-major so round db's weight slice is one contiguous DMA.

Schedule: one DMA issue stream on the Sync engine in consumption order
(issue cost ~650ns each serializes on Sync; transfers fan out over all 16
DMA engines at ~344 GB/s aggregate). V = x@Wv streams et-major right behind
the DMA. Then 8 rounds, one head-pair each: Q^T/K^T projections, previous
round's PV + epilogue, scores + exp chain (exp on ACT, -1 on GpSimd, mask
mul on DVE). Output projection runs as one dense block at the end from the
accumulated A^T tiles.
"""

import numpy as np
import ml_dtypes
from contextlib import ExitStack

import concourse.bass as bass
import concourse.tile as tile
from concourse import bacc, mybir
from concourse.bass_utils import run_bass_kernel_spmd

F32 = mybir.dt.float32
BF16 = mybir.dt.bfloat16
NPBF16 = ml_dtypes.bfloat16
N, E, H, D = 2048, 1024, 16, 64
R = N // 8          # 256 query rows per core
HALO = R + 128      # 384 K/V rows per core
NQB = R // 128      # query blocks per core


def build_graph():
    nc = bacc.Bacc("TRN2", target_bir_lowering=False, debug=False, num_devices=8)

    xT_d = nc.declare_dram_parameter("xT", [128, 8 * HALO], BF16, isOutput=False)
    wqk_d = nc.declare_dram_parameter("wqk", [E, 2048], BF16, isOutput=False)
    wv_d = nc.declare_dram_parameter("Wv", [E, H * D], BF16, isOutput=False)
    wo_d = nc.declare_dram_parameter("Wo", [H * D, E], BF16, isOutput=False)
    sm_d = nc.declare_dram_parameter("smalls", [128, 16], F32, isOutput=False)
    mb_d = nc.declare_dram_parameter("maskb", [128, 640], BF16, isOutput=False)
    bc_d = nc.declare_dram_parameter("biascat", [1, H * (D + 1)], BF16,
                                     isOutput=False)
    bo2_d = nc.declare_dram_parameter("bo2t", [128, E], F32, isOutput=False)
    out_d = nc.declare_dram_parameter("out", [R, E], F32, isOutput=True)

    with tile.TileContext(nc) as tc, ExitStack() as ctx:
        const = ctx.enter_context(tc.tile_pool(name="const", bufs=1))
        pers = ctx.enter_context(tc.tile_pool(name="pers", bufs=1))
        qtp = ctx.enter_context(tc.tile_pool(name="qtp", bufs=2))
        ktp = ctx.enter_context(tc.tile_pool(name="ktp", bufs=2))
        epool = ctx.enter_context(tc.tile_pool(name="epool", bufs=3))
        ppool = ctx.enter_context(tc.tile_pool(name="ppool", bufs=4))
        apool = ctx.enter_context(tc.tile_pool(name="apool", bufs=2))
        zpool = ctx.enter_context(tc.tile_pool(name="zpool", bufs=8))
        obpool = ctx.enter_context(tc.tile_pool(name="obpool", bufs=2))
        psum = ctx.enter_context(tc.tile_pool(name="psum", bufs=8, space="PSUM"))

        def ps(shape, dt=F32):
            return psum.tile(shape, dt, tag="ps", name="pst")

        # ---- SBUF tiles -------------------------------------------------
        xT = const.tile([128, 8, HALO], BF16, tag="xT")
        wqk_t = [const.tile([128, 2, 8, 128], BF16, tag=f"wqk{db}", name="wqkt")
                 for db in range(8)]
        wv_t = [const.tile([128, E], BF16, tag=f"wv{et}", name="wvt") for et in range(8)]
        wo_t = [const.tile([128, E], BF16, tag=f"wo{db}", name="wot") for db in range(8)]
        smalls = const.tile([128, 16], F32, tag="smalls")
        bq_sb = smalls[:, 0:8]
        bk_sb = smalls[:, 8:16]
        maskb = const.tile([128, 640], BF16, tag="maskb")
        m4 = maskb[:, 0:512]
        ident = maskb[:, 512:640]
        biascat = const.tile([1, H, D + 1], BF16, tag="biascat")
        bo2 = const.tile([128, E], F32, tag="bo2")
        ones_sb = const.tile([1, 128], BF16, tag="ones")
        nc.vector.memset(ones_sb[:], 1.0)

        Vaug = pers.tile([128, 3, H, D + 1], BF16, tag="Vaug")
        AT = pers.tile([128, 8, R], BF16, tag="AT")  # [d_p, db, q]

        # ---- single DMA issue stream on Sync, consumption order ---------
        for c in range(4):  # xT in 4 et-pair chunks
            nc.sync.dma_start(xT[:, 2 * c:2 * c + 2, :],
                              xT_d[:, 2 * c * HALO:(2 * c + 2) * HALO])
        for et in range(5):
            nc.sync.dma_start(wv_t[et][:], wv_d[et * 128:(et + 1) * 128, :])
        nc.sync.dma_start(wqk_t[0][:], wqk_d[0:128, :])
        for et in range(5, 8):
            nc.sync.dma_start(wv_t[et][:], wv_d[et * 128:(et + 1) * 128, :])
        nc.sync.dma_start(smalls[:], sm_d[:, :])
        nc.sync.dma_start(maskb[:], mb_d[:, :])
        nc.sync.dma_start(biascat[:], bc_d[:, :])
        for db in range(1, 8):
            nc.sync.dma_start(wqk_t[db][:], wqk_d[db * 128:(db + 1) * 128, :])
        for db in range(8):
            nc.sync.dma_start(wo_t[db][:], wo_d[db * 128:(db + 1) * 128, :])
        nc.sync.dma_start(bo2[:], bo2_d[:, :])

        # ---- PE clock warm-up during the x/wv DMA wait ------------------
        wu = const.tile([128, 128], BF16, tag="wu")
        nc.vector.memset(wu[:], 0.0)
        wups = psum.tile([128, 128], F32, tag="ps", name="wups")
        for _ in range(24):
            nc.tensor.matmul(wups[:], wu[:], wu[:], start=True, stop=True)

        # ---- V projection, et-major (streams behind the DMA) ------------
        vps = [psum.tile([128, 512], F32, tag="ps", name="vps") for _ in range(6)]  # st*2 + hf
        for et in range(8):
            for st in range(3):
                for hf in range(2):
                    nc.tensor.matmul(vps[st * 2 + hf][:],
                                     xT[:, et, st * 128:(st + 1) * 128],
                                     wv_t[et][:, hf * 512:(hf + 1) * 512],
                                     start=(et == 0), stop=(et == 7))
        for st in range(3):
            for hf in range(2):
                src = vps[st * 2 + hf][:].rearrange("p (h d) -> p h d", d=D)
                dst = Vaug[:, st, hf * 8:(hf + 1) * 8, 0:D]
                if hf == 0:
                    nc.scalar.copy(dst, src)
                else:
                    nc.vector.tensor_copy(dst, src)
        nc.vector.memset(Vaug[:, :, :, D:D + 1], 1.0)

        # ---- fused rounds: one head-pair db per round --------------------
        def pv_mms(pr):
            db, ptl = pr
            pv = ps([128, 2, 2, 65])  # [qblk, i, D+1]
            first = True
            for qblk in range(NQB):
                for i in range(2):
                    for cblk in range(2):
                        quad = qblk * 2 + cblk
                        nc.tensor.matmul(pv[:, qblk, i, :],
                                         ptl[i][:, quad * 128:(quad + 1) * 128],
                                         Vaug[:, qblk + cblk, 2 * db + i, :],
                                         start=first, stop=False)
                        first = False
            for qblk in range(NQB):
                for i in range(2):
                    nc.tensor.matmul(pv[:, qblk, i, :], ones_sb[0:1, :],
                                     biascat[0:1, 2 * db + i, :], start=False,
                                     stop=(qblk == 1 and i == 1))
            return pv

        def pv_epilogue(db, pv):
            asc = apool.tile([128, 2, 128], BF16, tag="asc", name="asc")
            for qblk in range(NQB):
                for i in range(2):
                    zin = zpool.tile([128, 1], F32, tag="z", name="zin")
                    nc.vector.reciprocal(zin[:], pv[:, qblk, i, 64:65])
                    nc.scalar.activation(asc[:, qblk, i * 64:(i + 1) * 64],
                                         pv[:, qblk, i, 0:64],
                                         mybir.ActivationFunctionType.Copy,
                                         scale=zin[:])
            return asc

        def at_transposes(db, asc):
            for qblk in range(NQB):
                tp = ps([128, 128], BF16)
                nc.tensor.transpose(tp[:], asc[:, qblk, :], ident[:])
                nc.vector.tensor_copy(AT[:, db, qblk * 128:(qblk + 1) * 128],
                                      tp[:])

        prev = None  # (db, ptiles)
        for r in range(9):
            if r < 8:
                db = r
                qp = ps([128, R])
                for et in range(8):
                    nc.tensor.matmul(qp[:], wqk_t[db][:, 0, et, :],
                                     xT[:, et, 64:64 + R],
                                     start=(et == 0), stop=(et == 7))
                qt = qtp.tile([128, R], BF16, tag="qt", name="qt")
                nc.scalar.add(qt[:], qp[:], bq_sb[:, db:db + 1])
                kp = ps([128, HALO])
                for et in range(8):
                    nc.tensor.matmul(kp[:], wqk_t[db][:, 1, et, :],
                                     xT[:, et, :],
                                     start=(et == 0), stop=(et == 7))
                kt = ktp.tile([128, HALO], BF16, tag="kt", name="kt")
                nc.scalar.add(kt[:], kp[:], bk_sb[:, db:db + 1])
                if prev is not None:
                    pvp = pv_mms(prev)
                    pasc = pv_epilogue(prev[0], pvp)
                ptl = {}
                for i in range(2):
                    rr = i * 64
                    sp = ps([128, 512])
                    for quad in range(4):
                        qblk, cblk = quad // 2, quad % 2
                        nc.tensor.matmul(
                            sp[:, quad * 128:(quad + 1) * 128],
                            kt[rr:rr + 64,
                               (qblk + cblk) * 128:(qblk + cblk + 1) * 128],
                            qt[rr:rr + 64, qblk * 128:(qblk + 1) * 128],
                            start=(quad == 0), stop=(quad == 3))
                    et_ = epool.tile([128, 512], BF16, tag="e", name="et_")
                    nc.scalar.activation(et_[:], sp[:],
                                         mybir.ActivationFunctionType.Exp)
                    nc.vector.tensor_scalar_add(et_[:], et_[:], -1.0)
                    pt = ppool.tile([128, 512], BF16, tag="p", name="pt")
                    nc.vector.tensor_mul(pt[:], et_[:], m4)
                    ptl[i] = pt
                if prev is not None:
                    at_transposes(prev[0], pasc)
                prev = (db, ptl)
            else:
                pvp = pv_mms(prev)
                pasc = pv_epilogue(prev[0], pvp)
                at_transposes(prev[0], pasc)

        # ---- output projection: dense block at the end -------------------
        # O[q, :] = sum_db AT[:, db, q]^T @ Wo[db*128:(db+1)*128, :] + bo2
        for qblk in range(NQB):
            ops = [psum.tile([128, 512], F32, tag="ps", name="ops") for _ in range(2)]
            for db in range(8):
                for hf in range(2):
                    nc.tensor.matmul(ops[hf][:],
                                     AT[:, db, qblk * 128:(qblk + 1) * 128],
                                     wo_t[db][:, hf * 512:(hf + 1) * 512],
                                     start=(db == 0), stop=(db == 7))
            ob = obpool.tile([128, E], F32, tag="ob", name="ob")
            for hf in range(2):
                nc.vector.tensor_tensor(ob[:, hf * 512:(hf + 1) * 512],
                                        ops[hf][:], bo2[:, hf * 512:(hf + 1) * 512],
                                        mybir.AluOpType.add)
            nc.sync.dma_start(out_d[qblk * 128:(qblk + 1) * 128, :], ob[:])

    nc.compile()
    return nc


_NC = None


def get_nc():
    global _NC
    if _NC is None:
        _NC = build_graph()
    return _NC


def make_in_maps(x, Wq, bq, Wk, bk, Wv, bv, Wo, bo):
    f = lambda a: np.ascontiguousarray(np.asarray(a, dtype=np.float32))
    bf = lambda a: np.ascontiguousarray(
        np.asarray(a, dtype=np.float32).astype(NPBF16))
    x2 = f(x).reshape(N, E)
    Wqf, Wkf, Wvf, Wof = f(Wq), f(Wk), f(Wv), f(Wo)
    # Wq/Wk column-block-major: wqk[db, p, qk, et, j] = W[et*128+p, db*128+j]
    wqT = Wqf.reshape(8, 128, 8, 128).transpose(2, 1, 0, 3)
    wkT = Wkf.reshape(8, 128, 8, 128).transpose(2, 1, 0, 3)
    wqk = np.stack([wqT, wkT], axis=2).reshape(E, 2048)
    # band masks per quadrant [m0 | m1 | m0 | m1]
    ci = np.arange(128, dtype=np.float32)[:, None]  # key index c (partitions)
    qi = np.arange(128, dtype=np.float32)[None, :]  # query index q (free)
    m0 = (ci >= qi).astype(np.float32)
    m1 = (ci <= qi).astype(np.float32)
    mask4 = np.concatenate([m0, m1, m0, m1], axis=1)
    # biascat rows: [SV_h (64) | N] per head, SV = (sum_n x_n) @ Wv
    SV = x2.sum(0, dtype=np.float32) @ Wvf
    bcat = np.zeros((H, D + 1), np.float32)
    bcat[:, 0:D] = SV.reshape(H, D)
    bcat[:, D] = float(N)
    bo2 = f(bv) @ Wof + f(bo)
    common = {
        "wqk": bf(wqk),
        "Wv": bf(Wvf), "Wo": bf(Wof),
        "biascat": bcat.reshape(1, H * (D + 1)).astype(NPBF16),
        "bo2t": np.ascontiguousarray(np.tile(bo2[None, :], (128, 1))),
    }
    in_maps = []
    for c in range(8):
        r0 = c * R
        xh = np.zeros((HALO, E), np.float32)
        lo, hi = r0 - 64, r0 + R + 64
        slo, shi = max(lo, 0), min(hi, N)
        xh[slo - lo: shi - lo] = x2[slo:shi]
        xTh = xh.T.reshape(8, 128, HALO).transpose(1, 0, 2).reshape(128, 8 * HALO)
        m4c = mask4.copy()
        if c == 0:    # halo rows 0:64 are padding, used only by quad 0
            m4c[0:64, 0:128] = 0.0
        if c == 7:    # halo rows 320:384 are padding, used only by quad 3
            m4c[64:128, 384:512] = 0.0
        sm = np.zeros((128, 16), np.float32)
        sm[:, 0:8] = f(bq).reshape(8, 128).T
        sm[:, 8:16] = f(bk).reshape(8, 128).T
        mb = np.zeros((128, 640), np.float32)
        mb[:, 0:512] = m4c
        mb[:, 512:640] = np.eye(128, dtype=np.float32)
        in_maps.append({**common,
                        "xT": np.ascontiguousarray(xTh.astype(NPBF16)),
                        "smalls": np.ascontiguousarray(sm),
                        "maskb": np.ascontiguousarray(mb.astype(NPBF16))})
    return in_maps


def kernel(x, Wq, bq, Wk, bk, Wv, bv, Wo, bo, _trace=False, _trace_kwargs=None):
    nc = get_nc()
    in_maps = make_in_maps(x, Wq, bq, Wk, bk, Wv, bv, Wo, bo)
    res = run_bass_kernel_spmd(nc, in_maps, list(range(8)), trace=_trace,
                               **(_trace_kwargs or {}))
    out = np.concatenate([res.results[c]["out"] for c in range(8)], axis=0)
    kernel.last_result = res
    return out[None].astype(np.float32)


# revision 7
# speedup vs baseline: 2.7236x; 1.2358x over previous
"""Multi-head dilated sliding-window attention (window=129, dil=1) on 8 TRN2 cores.

Sharding: sequence-parallel. Each core computes 256 query rows (N=2048 / 8),
with a 64-row K/V halo on each side (zero-padded at the sequence edges).
Weights are replicated (streamed to SBUF, bf16).

Band-softmax identity (reference softmaxes the FULL row with zeros outside
the band):
    out_i = (sum_band (e^{s_ij} - 1) V_j + sum_all V_j) / (sum_band (e^{s_ij} - 1) + N)
with V_raw = x@Wv (bv folded into bo2 = bv@Wo + bo on host), bk added as a
per-partition scalar on the K^T copy (padding keys are excluded by per-core
edge-masked band masks), and sum_all V_j = (sum_n x_n)@Wv computed on host.

Host-side layout prep: x arrives pre-transposed (xT[e,seq]), Wq/Wk arrive
column-block-major so round db's weight slice is one contiguous DMA.

Schedule: one DMA issue stream on the Sync engine in consumption order
(issue cost ~650ns each serializes on Sync; transfers fan out over all 16
DMA engines at ~344 GB/s aggregate). V = x@Wv streams et-major right behind
the DMA. Then 8 rounds, one head-pair each: Q^T/K^T projections, previous
round's PV + epilogue, scores + exp chain (exp on ACT, -1 on GpSimd, mask
mul on DVE). Output projection runs as one dense block at the end from the
accumulated A^T tiles.
"""

import numpy as np
import ml_dtypes
from contextlib import ExitStack

import concourse.bass as bass
import concourse.tile as tile
from concourse import bacc, mybir
from concourse.bass_utils import run_bass_kernel_spmd

F32 = mybir.dt.float32
BF16 = mybir.dt.bfloat16
NPBF16 = ml_dtypes.bfloat16
N, E, H, D = 2048, 1024, 16, 64
R = N // 8          # 256 query rows per core
HALO = R + 128      # 384 K/V rows per core
NQB = R // 128      # query blocks per core


def build_graph():
    nc = bacc.Bacc("TRN2", target_bir_lowering=False, debug=False, num_devices=8)

    xT_d = nc.declare_dram_parameter("xT", [128, 8 * HALO], BF16, isOutput=False)
    wqk_d = nc.declare_dram_parameter("wqk", [E, 2048], BF16, isOutput=False)
    wv_d = nc.declare_dram_parameter("Wv", [E, H * D], BF16, isOutput=False)
    wo_d = nc.declare_dram_parameter("Wo", [H * D, E], BF16, isOutput=False)
    sm_d = nc.declare_dram_parameter("smalls", [128, 16], F32, isOutput=False)
    mb_d = nc.declare_dram_parameter("maskb", [128, 640], BF16, isOutput=False)
    bc_d = nc.declare_dram_parameter("biascat", [1, H * (D + 1)], BF16,
                                     isOutput=False)
    bo2_d = nc.declare_dram_parameter("bo2t", [128, E], F32, isOutput=False)
    out_d = nc.declare_dram_parameter("out", [R, E], F32, isOutput=True)

    with tile.TileContext(nc) as tc, ExitStack() as ctx:
        const = ctx.enter_context(tc.tile_pool(name="const", bufs=1))
        pers = ctx.enter_context(tc.tile_pool(name="pers", bufs=1))
        qtp = ctx.enter_context(tc.tile_pool(name="qtp", bufs=2))
        ktp = ctx.enter_context(tc.tile_pool(name="ktp", bufs=2))
        epool = ctx.enter_context(tc.tile_pool(name="epool", bufs=3))
        ppool = ctx.enter_context(tc.tile_pool(name="ppool", bufs=4))
        apool = ctx.enter_context(tc.tile_pool(name="apool", bufs=2))
        zpool = ctx.enter_context(tc.tile_pool(name="zpool", bufs=8))
        obpool = ctx.enter_context(tc.tile_pool(name="obpool", bufs=2))
        psum = ctx.enter_context(tc.tile_pool(name="psum", bufs=8, space="PSUM"))

        def ps(shape, dt=F32):
            return psum.tile(shape, dt, tag="ps", name="pst")

        # ---- SBUF tiles -------------------------------------------------
        xT = const.tile([128, 8, HALO], BF16, tag="xT")
        wqk_t = [const.tile([128, 2, 8, 128], BF16, tag=f"wqk{db}", name="wqkt")
                 for db in range(8)]
        wv_t = [const.tile([128, E], BF16, tag=f"wv{et}", name="wvt") for et in range(8)]
        wo_t = [const.tile([128, E], BF16, tag=f"wo{db}", name="wot") for db in range(8)]
        smalls = const.tile([128, 16], F32, tag="smalls")
        bq_sb = smalls[:, 0:8]
        bk_sb = smalls[:, 8:16]
        maskb = const.tile([128, 640], BF16, tag="maskb")
        m4 = maskb[:, 0:512]
        ident = maskb[:, 512:640]
        biascat = const.tile([1, H, D + 1], BF16, tag="biascat")
        bo2 = const.tile([128, E], F32, tag="bo2")
        ones_sb = const.tile([1, 128], BF16, tag="ones")
        nc.vector.memset(ones_sb[:], 1.0)

        Vaug = pers.tile([128, 3, H, D + 1], BF16, tag="Vaug")
        AT = pers.tile([128, 8, R], BF16, tag="AT")  # [d_p, db, q]

        # ---- single DMA issue stream on Sync, consumption order ---------
        for c in range(4):  # xT in 4 et-pair chunks
            nc.sync.dma_start(xT[:, 2 * c:2 * c + 2, :],
                              xT_d[:, 2 * c * HALO:(2 * c + 2) * HALO])
        for et in range(5):
            nc.sync.dma_start(wv_t[et][:], wv_d[et * 128:(et + 1) * 128, :])
        nc.sync.dma_start(wqk_t[0][:], wqk_d[0:128, :])
        for et in range(5, 8):
            nc.sync.dma_start(wv_t[et][:], wv_d[et * 128:(et + 1) * 128, :])
        nc.sync.dma_start(smalls[:], sm_d[:, :])
        nc.sync.dma_start(maskb[:], mb_d[:, :])
        nc.sync.dma_start(biascat[:], bc_d[:, :])
        for db in range(1, 8):
            nc.sync.dma_start(wqk_t[db][:], wqk_d[db * 128:(db + 1) * 128, :])
        for db in range(8):
            nc.sync.dma_start(wo_t[db][:], wo_d[db * 128:(db + 1) * 128, :])
        nc.sync.dma_start(bo2[:], bo2_d[:, :])

        # ---- PE clock warm-up during the x/wv DMA wait ------------------
        wu = const.tile([128, 128], BF16, tag="wu")
        nc.vector.memset(wu[:], 0.0)
        wups = psum.tile([128, 128], F32, tag="ps", name="wups")
        for _ in range(24):
            nc.tensor.matmul(wups[:], wu[:], wu[:], start=True, stop=True)

        # ---- V projection, et-major (streams behind the DMA) ------------
        vps = [psum.tile([128, 512], F32, tag="ps", name="vps") for _ in range(6)]  # st*2 + hf
        for et in range(8):
            for st in range(3):
                for hf in range(2):
                    nc.tensor.matmul(vps[st * 2 + hf][:],
                                     xT[:, et, st * 128:(st + 1) * 128],
                                     wv_t[et][:, hf * 512:(hf + 1) * 512],
                                     start=(et == 0), stop=(et == 7))
        for st in range(3):
            for hf in range(2):
                src = vps[st * 2 + hf][:].rearrange("p (h d) -> p h d", d=D)
                dst = Vaug[:, st, hf * 8:(hf + 1) * 8, 0:D]
                if hf == 0:
                    nc.scalar.copy(dst, src)
                else:
                    nc.vector.tensor_copy(dst, src)
        nc.vector.memset(Vaug[:, :, :, D:D + 1], 1.0)

        # ---- fused rounds: one head-pair db per round --------------------
        def pv_mms(pr):
            db, ptl = pr
            pvs = []
            for qblk in range(NQB):
                pv = ps([128, 2, 65])  # [i, D+1]
                for i in range(2):
                    for cblk in range(2):
                        quad = qblk * 2 + cblk
                        nc.tensor.matmul(pv[:, i, :],
                                         ptl[i][:, quad * 128:(quad + 1) * 128],
                                         Vaug[:, qblk + cblk, 2 * db + i, :],
                                         start=(i == 0 and cblk == 0), stop=False)
                for i in range(2):
                    nc.tensor.matmul(pv[:, i, :], ones_sb[0:1, :],
                                     biascat[0:1, 2 * db + i, :], start=False,
                                     stop=(i == 1))
                pvs.append(pv)
            return pvs

        def pv_epilogue(db, pvs):
            asc = apool.tile([128, 2, 128], BF16, tag="asc", name="asc")
            for qblk in range(NQB):
                for i in range(2):
                    zin = zpool.tile([128, 1], F32, tag="z", name="zin")
                    nc.vector.reciprocal(zin[:], pvs[qblk][:, i, 64:65])
                    nc.vector.tensor_scalar_mul(
                        asc[:, qblk, i * 64:(i + 1) * 64],
                        pvs[qblk][:, i, 0:64], zin[:])
            return asc

        def at_transposes(db, asc):
            for qblk in range(NQB):
                tp = ps([128, 128], BF16)
                nc.tensor.transpose(tp[:], asc[:, qblk, :], ident[:])
                nc.scalar.copy(AT[:, db, qblk * 128:(qblk + 1) * 128], tp[:])

        prev = None  # (db, ptiles)
        for r in range(9):
            if r < 8:
                db = r
                qp = ps([128, R])
                for et in range(8):
                    nc.tensor.matmul(qp[:], wqk_t[db][:, 0, et, :],
                                     xT[:, et, 64:64 + R],
                                     start=(et == 0), stop=(et == 7))
                qt = qtp.tile([128, R], BF16, tag="qt", name="qt")
                nc.scalar.add(qt[:], qp[:], bq_sb[:, db:db + 1])
                kp = ps([128, HALO])
                for et in range(8):
                    nc.tensor.matmul(kp[:], wqk_t[db][:, 1, et, :],
                                     xT[:, et, :],
                                     start=(et == 0), stop=(et == 7))
                kt = ktp.tile([128, HALO], BF16, tag="kt", name="kt")
                nc.scalar.add(kt[:], kp[:], bk_sb[:, db:db + 1])
                if prev is not None:
                    pvp = pv_mms(prev)
                    pasc = pv_epilogue(prev[0], pvp)
                ptl = {}
                for i in range(2):
                    rr = i * 64
                    sp = ps([128, 512])
                    for quad in range(4):
                        qblk, cblk = quad // 2, quad % 2
                        nc.tensor.matmul(
                            sp[:, quad * 128:(quad + 1) * 128],
                            kt[rr:rr + 64,
                               (qblk + cblk) * 128:(qblk + cblk + 1) * 128],
                            qt[rr:rr + 64, qblk * 128:(qblk + 1) * 128],
                            start=(quad == 0), stop=(quad == 3))
                    et_ = epool.tile([128, 512], BF16, tag="e", name="et_")
                    nc.scalar.activation(et_[:], sp[:],
                                         mybir.ActivationFunctionType.Exp)
                    nc.vector.tensor_scalar_add(et_[:], et_[:], -1.0)
                    pt = ppool.tile([128, 512], BF16, tag="p", name="pt")
                    nc.vector.tensor_mul(pt[:], et_[:], m4)
                    ptl[i] = pt
                if prev is not None:
                    at_transposes(prev[0], pasc)
                prev = (db, ptl)
            else:
                pvp = pv_mms(prev)
                pasc = pv_epilogue(prev[0], pvp)
                at_transposes(prev[0], pasc)

        # ---- output projection: dense block at the end -------------------
        # O[q, :] = sum_db AT[:, db, q]^T @ Wo[db*128:(db+1)*128, :] + bo2
        for qblk in range(NQB):
            ops = [psum.tile([128, 512], F32, tag="ps", name="ops") for _ in range(2)]
            for db in range(8):
                for hf in range(2):
                    nc.tensor.matmul(ops[hf][:],
                                     AT[:, db, qblk * 128:(qblk + 1) * 128],
                                     wo_t[db][:, hf * 512:(hf + 1) * 512],
                                     start=(db == 0), stop=(db == 7))
            ob = obpool.tile([128, E], F32, tag="ob", name="ob")
            for hf in range(2):
                nc.vector.tensor_tensor(ob[:, hf * 512:(hf + 1) * 512],
                                        ops[hf][:], bo2[:, hf * 512:(hf + 1) * 512],
                                        mybir.AluOpType.add)
            nc.sync.dma_start(out_d[qblk * 128:(qblk + 1) * 128, :], ob[:])

    nc.compile()
    return nc


_NC = None


def get_nc():
    global _NC
    if _NC is None:
        _NC = build_graph()
    return _NC


def make_in_maps(x, Wq, bq, Wk, bk, Wv, bv, Wo, bo):
    f = lambda a: np.ascontiguousarray(np.asarray(a, dtype=np.float32))
    bf = lambda a: np.ascontiguousarray(
        np.asarray(a, dtype=np.float32).astype(NPBF16))
    x2 = f(x).reshape(N, E)
    Wqf, Wkf, Wvf, Wof = f(Wq), f(Wk), f(Wv), f(Wo)
    # Wq/Wk column-block-major: wqk[db, p, qk, et, j] = W[et*128+p, db*128+j]
    wqT = Wqf.reshape(8, 128, 8, 128).transpose(2, 1, 0, 3)
    wkT = Wkf.reshape(8, 128, 8, 128).transpose(2, 1, 0, 3)
    wqk = np.stack([wqT, wkT], axis=2).reshape(E, 2048)
    # band masks per quadrant [m0 | m1 | m0 | m1]
    ci = np.arange(128, dtype=np.float32)[:, None]  # key index c (partitions)
    qi = np.arange(128, dtype=np.float32)[None, :]  # query index q (free)
    m0 = (ci >= qi).astype(np.float32)
    m1 = (ci <= qi).astype(np.float32)
    mask4 = np.concatenate([m0, m1, m0, m1], axis=1)
    # biascat rows: [SV_h (64) | N] per head, SV = (sum_n x_n) @ Wv
    SV = x2.sum(0, dtype=np.float32) @ Wvf
    bcat = np.zeros((H, D + 1), np.float32)
    bcat[:, 0:D] = SV.reshape(H, D)
    bcat[:, D] = float(N)
    bo2 = f(bv) @ Wof + f(bo)
    common = {
        "wqk": bf(wqk),
        "Wv": bf(Wvf), "Wo": bf(Wof),
        "biascat": bcat.reshape(1, H * (D + 1)).astype(NPBF16),
        "bo2t": np.ascontiguousarray(np.tile(bo2[None, :], (128, 1))),
    }
    in_maps = []
    for c in range(8):
        r0 = c * R
        xh = np.zeros((HALO, E), np.float32)
        lo, hi = r0 - 64, r0 + R + 64
        slo, shi = max(lo, 0), min(hi, N)
        xh[slo - lo: shi - lo] = x2[slo:shi]
        xTh = xh.T.reshape(8, 128, HALO).transpose(1, 0, 2).reshape(128, 8 * HALO)
        m4c = mask4.copy()
        if c == 0:    # halo rows 0:64 are padding, used only by quad 0
            m4c[0:64, 0:128] = 0.0
        if c == 7:    # halo rows 320:384 are padding, used only by quad 3
            m4c[64:128, 384:512] = 0.0
        sm = np.zeros((128, 16), np.float32)
        sm[:, 0:8] = f(bq).reshape(8, 128).T
        sm[:, 8:16] = f(bk).reshape(8, 128).T
        mb = np.zeros((128, 640), np.float32)
        mb[:, 0:512] = m4c
        mb[:, 512:640] = np.eye(128, dtype=np.float32)
        in_maps.append({**common,
                        "xT": np.ascontiguousarray(xTh.astype(NPBF16)),
                        "smalls": np.ascontiguousarray(sm),
                        "maskb": np.ascontiguousarray(mb.astype(NPBF16))})
    return in_maps


def kernel(x, Wq, bq, Wk, bk, Wv, bv, Wo, bo, _trace=False, _trace_kwargs=None):
    nc = get_nc()
    in_maps = make_in_maps(x, Wq, bq, Wk, bk, Wv, bv, Wo, bo)
    res = run_bass_kernel_spmd(nc, in_maps, list(range(8)), trace=_trace,
                               **(_trace_kwargs or {}))
    out = np.concatenate([res.results[c]["out"] for c in range(8)], axis=0)
    kernel.last_result = res
    return out[None].astype(np.float32)
